# revision 9
# baseline (speedup 1.0000x reference)
"""Trainium2 Bass kernel for the HGCA contrastive loss (nn_HGCA_10857677324785).

loss = mean over i of 0.5*(l1_i + l2_i) where
  h1 = elu(z1@W1+b1)@W2+b2 ; h2 likewise ; an, bn = l2-normalized rows
  l1_i = -log( exp(an_i.bn_i/tau) / (sum_j exp(an_i.an_j/tau)
               + sum_j exp(an_i.bn_j/tau) - e^{1/tau}) )
  l2_i symmetric with row sums of exp(bn@bn.T) and exp(bn@an.T).

Distribution: rows sharded over 8 cores. Host rolls z1/z2 per core so each
core's row block sits at local rows [0,2048). Each core computes the full
normalized projections (cheap, O(N D^2)), then its row-block of the three
N x N similarity matrices flash-style: exp row sums on ACT (fused accum),
plus per-column partial sums of exp(an@bn.T) (for l2's "between" term, which
equals column sums of the l1 "between" matrix). Host assembles the scalar
loss from O(N) partial sums.
"""

import re

import ml_dtypes
import numpy as np

import concourse.bass as bass
import concourse.tile as tile
from concourse import mybir
from concourse.bass_utils import run_bass_kernel_spmd
from concourse.masks import make_identity
from concourse.vector_clock import ScopedClock, VectorClock

N = 16384
D = 128
NCORES = 8
R = N // NCORES  # 2048 rows per core
INV_TAU = 2.0  # 1/0.5
F32 = mybir.dt.float32
BF16 = mybir.dt.bfloat16
AF = mybir.ActivationFunctionType
OP = mybir.AluOpType

# This walrus build supports at most 2 sync waits per instruction; Tile's sem
# assignment freely emits 3-11. Post-pass: hoist excess waits onto injected
# same-engine EventSemaphore fillers (engine queues are FIFO, so waits on an
# earlier filler happen-before the original instruction executes).

_MAX_WAITS = 1


def _split_waits(nc):
    for fn in nc.m.functions:
        for bb in fn.blocks:
            insts = list(bb.instructions)
            out = []
            changed = False
            for inst in insts:
                si = inst.sync_info
                w = list(si.on_wait) if si and si.on_wait else []
                if len(w) > _MAX_WAITS:
                    changed = True
                    extra, keep = w[:-_MAX_WAITS], w[-_MAX_WAITS:]
                    for i in range(0, len(extra), _MAX_WAITS):
                        f = mybir.InstEventSemaphore(
                            name=f"{inst.name}_wsplit{i}",
                            engine=inst.engine,
                            ins=[],
                            outs=[],
                            sync_info=mybir.SyncInfo(
                                on_wait=extra[i : i + _MAX_WAITS], on_update=[]
                            ),
                        )
                        out.append(f)
                    inst.sync_info = mybir.SyncInfo(
                        on_wait=keep,
                        on_update=list(si.on_update) if si.on_update else [],
                    )
                out.append(inst)
            if changed:
                bb.instructions = out


def _patched_drain_and_barrier(self, tick_clock, wait_clock):
    nc = self.nc
    drain_inst = nc.sync.drain()
    wait_clock.add_sem_waits(
        drain_inst.ins, ScopedClock({None: tick_clock.global_clock})
    )
    nc.all_engine_barrier()
    assert self.sems is not None
    popped = nc._tile_sem_poison_stack.pop()
    assert popped is self._sem_poison
    nc.clear_and_free_semaphores(list(self.sems.allocated().values()))
    nc.all_engine_barrier()
    _split_waits(nc)


tile.TileContext._drain_and_barrier = _patched_drain_and_barrier

_NC_CACHE = None
RUN_KWARGS: dict = {}
LAST_RES = None


def _build():
    nc = bass.Bass("TRN2", target_bir_lowering=False, debug=False)

    z1_d = nc.dram_tensor("z1", [N, D], BF16, kind="ExternalInput").ap()
    z2_d = nc.dram_tensor("z2", [N, D], BF16, kind="ExternalInput").ap()
    w1_d = nc.dram_tensor("w1", [D, D], BF16, kind="ExternalInput").ap()
    w2_d = nc.dram_tensor("w2", [D, D], BF16, kind="ExternalInput").ap()
    b1_d = nc.dram_tensor("b1", [D, 1], F32, kind="ExternalInput").ap()
    b2p_d = nc.dram_tensor("b2p", [D, 1], F32, kind="ExternalInput").ap()

    rs_d = [
        nc.dram_tensor(f"rs{i}", [128, 16], F32, kind="ExternalOutput").ap()
        for i in range(3)
    ]
    cs12_d = nc.dram_tensor("cs12", [1, N], F32, kind="ExternalOutput").ap()
    num_d = nc.dram_tensor("num", [1, R], F32, kind="ExternalOutput").ap()

    with tile.TileContext(nc) as tc:
        with (
            tc.tile_pool(name="persist", bufs=1) as pers,
            tc.tile_pool(name="consts", bufs=1) as consts,
        ):
            anT = pers.tile([128, N], BF16, tag="anT")
            bnT = pers.tile([128, N], BF16, tag="bnT")
            rs_sb = [
                pers.tile([128, 16], F32, tag=f"rs{i}", name=f"rs_sb{i}")
                for i in range(3)
            ]

            ident = consts.tile([128, 128], BF16, tag="ident")
            make_identity(nc, ident[:])
            ones_col_bf = consts.tile([128, 1], BF16, tag="ocb")
            nc.gpsimd.memset(ones_col_bf[:], 1.0)
            ones_col_f = consts.tile([128, 1], F32, tag="ocf")
            nc.gpsimd.memset(ones_col_f[:], 1.0)
            ones_row_f = consts.tile([1, 128], F32, tag="orf")
            nc.gpsimd.memset(ones_row_f[:], 1.0)
            w1sb = consts.tile([128, 128], BF16, tag="w1")
            nc.sync.dma_start(w1sb[:], w1_d[:])
            w2sb = consts.tile([128, 128], BF16, tag="w2")
            nc.sync.dma_start(w2sb[:], w2_d[:])
            b1sb = consts.tile([128, 1], F32, tag="b1")
            nc.sync.dma_start(b1sb[:], b1_d[:])
            b2psb = consts.tile([128, 1], F32, tag="b2p")
            nc.sync.dma_start(b2psb[:], b2p_d[:])

            # ---------------- setup: projections + normalize ----------------
            with (
                tc.tile_pool(name="szt", bufs=2) as szt,
                tc.tile_pool(name="sw", bufs=4) as sw,
                tc.tile_pool(name="sp2", bufs=2, space="PSUM") as sp2,
                tc.tile_pool(name="sp1", bufs=1, space="PSUM") as sp1,
            ):
                for t, (z_d, aT) in enumerate([(z1_d, anT), (z2_d, bnT)]):
                    zT = szt.tile([128, N], BF16, tag="zT")
                    # transpose z into [d, i] layout via PE
                    for i in range(N // 128):
                        nat = sw.tile([128, 128], BF16, tag="nat")
                        nc.sync.dma_start(nat[:], z_d[i * 128 : (i + 1) * 128, :])
                        tps = sp1.tile([128, 128], BF16, tag="tps")
                        nc.tensor.transpose(tps[:], nat[:], ident[:])
                        nc.vector.tensor_copy(zT[:, i * 128 : (i + 1) * 128], tps[:])
                    # project + normalize, 512-wide chunks
                    for k in range(N // 512):
                        sl = slice(k * 512, (k + 1) * 512)
                        psA = sp2.tile([128, 512], F32, tag="psA")
                        nc.tensor.matmul(psA[:], w1sb[:], zT[:, sl])
                        expu = sw.tile([128, 512], F32, tag="expu")
                        nc.scalar.activation(expu[:], psA[:], AF.Exp, bias=b1sb[:])
                        relu = sw.tile([128, 512], F32, tag="relu")
                        nc.scalar.activation(relu[:], psA[:], AF.Relu, bias=b1sb[:])
                        # elu(y)+1 = min(exp(y),1) + max(y,0)
                        p1c = sw.tile([128, 512], BF16, tag="p1c")
                        nc.vector.scalar_tensor_tensor(
                            p1c[:], expu[:], 1.0, relu[:], OP.min, OP.add
                        )
                        psB = sp2.tile([128, 512], F32, tag="psB")
                        nc.tensor.matmul(psB[:], w2sb[:], p1c[:])
                        hc = sw.tile([128, 512], BF16, tag="hc")
                        nc.vector.tensor_scalar(hc[:], psB[:], b2psb[:], None, OP.add)
                        sq = sw.tile([128, 512], BF16, tag="sq")
                        nc.vector.tensor_mul(sq[:], hc[:], hc[:])
                        psC = sp1.tile([1, 512], F32, tag="psC")
                        nc.tensor.matmul(psC[:], ones_col_bf[:], sq[:])
                        lnq = sw.tile([1, 512], F32, tag="lnq")
                        nc.scalar.activation(lnq[:], psC[:], AF.Ln)
                        psD = sp2.tile([128, 512], F32, tag="psD")
                        nc.tensor.matmul(psD[:], ones_row_f[:], lnq[:])
                        invnb = sw.tile([128, 512], F32, tag="invnb")
                        nc.scalar.activation(invnb[:], psD[:], AF.Exp, scale=-0.5)
                        nc.vector.tensor_mul(aT[:, sl], invnb[:], hc[:])

                # num_i = exp(an_i . bn_i / tau) for local rows (cols 0..R)
                for q in range(R // 512):
                    sl = slice(q * 512, (q + 1) * 512)
                    prod = sw.tile([128, 512], F32, tag="prod")
                    nc.vector.tensor_mul(prod[:], anT[:, sl], bnT[:, sl])
                    psN = sp1.tile([1, 512], F32, tag="psC")
                    nc.tensor.matmul(psN[:], ones_col_f[:], prod[:])
                    numt = sw.tile([1, 512], F32, tag="numt")
                    nc.scalar.activation(numt[:], psN[:], AF.Exp, scale=INV_TAU)
                    nc.sync.dma_start(num_d[0:1, sl], numt[:])

            # ---------------- main loop: 3 similarity row-blocks ------------
            with tc.tile_pool(name="mp", bufs=1) as mp:
                colacc = mp.tile([128, N], F32, tag="colacc")
                nc.gpsimd.memset(colacc[:], 0.0)
                with (
                    tc.tile_pool(name="me", bufs=4) as me,
                    tc.tile_pool(name="ma", bufs=4) as ma,
                    tc.tile_pool(name="mpp", bufs=2, space="PSUM") as mpp,
                ):
                    mats = [(anT, anT, False), (anT, bnT, True), (bnT, bnT, False)]
                    for mi, (lhs, rhs, need_col) in enumerate(mats):
                        for m in range(R // 128):
                            lT = lhs[:, m * 128 : (m + 1) * 128]
                            acc8 = ma.tile([128, 8], F32, tag="acc8")
                            for jt in range(8):
                                ps = mpp.tile([128, 2048], F32, tag="mm")
                                for q in range(4):
                                    nc.tensor.matmul(
                                        ps[:, q * 512 : (q + 1) * 512],
                                        lT,
                                        rhs[:, jt * 2048 + q * 512 : jt * 2048 + (q + 1) * 512],
                                    )
                                E = me.tile([128, 2048], BF16, tag="E")
                                nc.scalar.activation(
                                    E[:],
                                    ps[:],
                                    AF.Exp,
                                    scale=INV_TAU,
                                    accum_out=acc8[:, jt : jt + 1],
                                )
                                if need_col:
                                    csl = slice(jt * 2048, (jt + 1) * 2048)
                                    nc.vector.scalar_tensor_tensor(
                                        colacc[:, csl], E[:], 1.0, colacc[:, csl],
                                        OP.mult, OP.add,
                                    )
                            nc.vector.tensor_reduce(
                                rs_sb[mi][:, m : m + 1], acc8[:],
                                mybir.AxisListType.X, OP.add,
                            )
                        nc.sync.dma_start(rs_d[mi][:], rs_sb[mi][:])

                # cs12[j] = sum over this core's rows of exp(S12)[.,j]
                with (
                    tc.tile_pool(name="cw", bufs=2) as cw,
                    tc.tile_pool(name="cpp", bufs=2, space="PSUM") as cpp,
                ):
                    for k in range(N // 512):
                        sl = slice(k * 512, (k + 1) * 512)
                        psK = cpp.tile([1, 512], F32, tag="psK")
                        nc.tensor.matmul(psK[:], ones_col_f[:], colacc[:, sl])
                        cst = cw.tile([1, 512], F32, tag="cst")
                        nc.vector.tensor_copy(cst[:], psK[:])
                        nc.sync.dma_start(cs12_d[0:1, sl], cst[:])

    return nc


def _get_nc():
    global _NC_CACHE
    if _NC_CACHE is None:
        _NC_CACHE = _build()
    return _NC_CACHE


def kernel(z1, z2, W1, b1, W2, b2):
    global LAST_RES
    bf = ml_dtypes.bfloat16
    z1 = np.asarray(z1, dtype=np.float32)
    z2 = np.asarray(z2, dtype=np.float32)
    W1 = np.asarray(W1, dtype=np.float32)
    W2 = np.asarray(W2, dtype=np.float32)
    b1 = np.asarray(b1, dtype=np.float32)
    b2 = np.asarray(b2, dtype=np.float32)
    # fold the "-1" of elu(y) = (min(exp y,1)+max(y,0)) - 1 into the 2nd bias
    b2p = (b2.astype(np.float64) - W2.astype(np.float64).sum(0)).astype(np.float32)

    nc = _get_nc()
    in_maps = []
    for c in range(NCORES):
        in_maps.append(
            {
                "z1": np.roll(z1, -c * R, axis=0).astype(bf),
                "z2": np.roll(z2, -c * R, axis=0).astype(bf),
                "w1": W1.astype(bf),
                "w2": W2.astype(bf),
                "b1": b1.reshape(D, 1).copy(),
                "b2p": b2p.reshape(D, 1).copy(),
            }
        )
    res = run_bass_kernel_spmd(nc, in_maps, list(range(NCORES)), **RUN_KWARGS)
    LAST_RES = res

    e2 = np.exp(np.float64(INV_TAU))
    rs11 = np.empty(N, np.float64)
    rs12 = np.empty(N, np.float64)
    rs22 = np.empty(N, np.float64)
    num = np.empty(N, np.float64)
    cs12 = np.zeros(N, np.float64)
    for c in range(NCORES):
        r = res.results[c]
        sl = slice(c * R, (c + 1) * R)
        rs11[sl] = r["rs0"].astype(np.float64).T.reshape(R)
        rs12[sl] = r["rs1"].astype(np.float64).T.reshape(R)
        rs22[sl] = r["rs2"].astype(np.float64).T.reshape(R)
        num[sl] = r["num"].astype(np.float64).reshape(R)
        cs12 += np.roll(r["cs12"].astype(np.float64).reshape(N), c * R)

    den1 = rs11 + rs12 - e2
    den2 = rs22 + cs12 - e2
    l1 = np.log(den1) - np.log(num)
    l2 = np.log(den2) - np.log(num)
    loss = np.mean(0.5 * (l1 + l2))
    return np.array(loss, dtype=np.float32)
